# revision 1
# baseline (speedup 1.0000x reference)
"""Trainium2 Bass kernel for EnhancedGradedLoss (Huber + pairwise hinge ranking).

Algorithm (see reference): loss = 0.7 * SmoothL1(p, t) + 0.3 * ranking, where
ranking averages relu(1 - sign(t_i - t_j) * (p_i - p_j)) over i<j pairs with
t_i != t_j.

Device strategy (8 NeuronCores, SPMD):
  * Targets take a small discrete set of grades. Sort items by grade on host
    (O(n) prep). Every unordered pair (a, b) with grade(a) > grade(b)
    contributes relu(1 - p_a + p_b); equal-grade pairs contribute nothing.
  * For each grade level h below the top, "window h" pairs every row with
    grade > h against the columns of grade h. Rows are sharded across the 8
    cores (padded with dead rows that contribute exactly zero); the sorted
    prediction vector B is replicated to all 128 SBUF partitions per core
    via a stride-0 broadcast DMA (bf16).
  * Each [128 rows x n_h cols] tile is ONE fused instruction:
      - ScalarE: activation(Relu, bias=c_row, accum_out) -> sum relu(B + c)
      - VectorE: tensor_scalar(max, scalar1=-c_row, accum_out) at 4x bf16
        -> sum max(B, -c) == sum relu(B + c) - n_h * c   (host adds n_h * c)
    Work is split across both engines to balance their modeled busy time.
  * Huber = 0.5*d^2 - 0.5*relu(d-1)^2 - 0.5*relu(-d-1)^2, sharded 1/8 per
    core; VectorE preps d/relu terms, ScalarE squares+accumulates. This runs
    during the broadcast-DMA head so it is effectively free.
  * Raw Bass program (no Tile framework): explicit per-engine instruction
    streams with hand-placed semaphores; per-core differences are carried
    entirely by input data, so one SPMD program serves all 8 cores.
  * Device outputs are per-partition accumulators only ([128, ~16] per
    core); the host reduces them in float64 and applies the closed-form
    dead-row/max-trick corrections.
"""

import functools
import sys

import ml_dtypes
import numpy as np

sys.path.insert(0, "/opt/trn_rl_repo")

import concourse.bacc as bacc
import concourse.bass as bass
from concourse import mybir
from concourse.bass_utils import run_bass_kernel_spmd

ALPHA = 0.7
BETA = 0.3
W0_CHUNKS = 2  # window-0 broadcast head split (earlier compute start)
NCORES = 8
P = 128  # SBUF partitions


def _plan(targets_f, predictions_f):
    """Host-side planning: sort by grade, window layout, per-core row shards."""
    n = targets_f.shape[0]
    order = np.argsort(targets_f, kind="stable")
    ts = targets_f[order]
    ps = predictions_f[order].astype(np.float32)

    # grade level boundaries (targets take a small discrete set of values)
    levels, counts = np.unique(ts, return_counts=True)
    K = len(levels)
    offs = np.concatenate([[0], np.cumsum(counts)]).astype(np.int64)  # len K+1

    pmax = float(np.max(np.abs(ps))) if n else 0.0
    dead = -float(np.float32(np.ceil(pmax) + 2.0))

    # columns: all grades except the top one
    L = int(offs[K - 1]) if K >= 2 else 0
    bcols = ps[:L].copy()

    # c values for every sorted row: c = 1 - p  (float32 exactly as device uses)
    c_all = (np.float32(1.0) - ps).astype(np.float32)

    # windows: h = 0..K-2; cols = [offs[h], offs[h+1]); rows = positions >= offs[h+1]
    windows = []
    for h in range(K - 1):
        col0 = int(offs[h])
        ncol = int(offs[h + 1] - offs[h])
        row0 = int(offs[h + 1])
        m = n - row0
        if ncol == 0 or m == 0:
            continue
        q = -(-m // NCORES)  # ceil: rows per core
        t = -(-q // P)  # tiles per core
        windows.append(dict(col0=col0, ncol=ncol, row0=row0, m=m, q=q, T=t))

    # per-core row-constant arrays (window-major, each window padded to T*128)
    cp_cores = []
    for c in range(NCORES):
        parts = []
        for w in windows:
            r0 = w["row0"] + c * w["q"]
            r1 = min(w["row0"] + min((c + 1) * w["q"], w["m"]), n)
            r0 = min(r0, r1)
            vals = c_all[r0:r1]
            padded = np.full(w["T"] * P, dead, dtype=np.float32)
            padded[: len(vals)] = vals
            parts.append(padded)
        cp_cores.append(
            np.concatenate(parts) if parts else np.zeros(0, dtype=np.float32)
        )

    # engine assignment per (window, tile): balance modeled busy-ns.
    # Iterate in window (data-arrival) order so each engine's early tiles
    # come from the earliest-DMA'd window and neither engine stalls on a
    # later window's broadcast.
    tiles = []
    for wi, w in enumerate(windows):
        for tj in range(w["T"]):
            tiles.append((wi, tj, w["ncol"]))
    t_dve = 0.0
    t_act = 0.0
    assign = {}
    for wi, tj, ncol in tiles:
        cost_d = (58.0 + ncol / 4.0) / 0.96 + 45.0
        cost_a = (224.0 + ncol) / 1.2 + 190.0
        if t_dve + cost_d <= t_act + cost_a:
            assign[(wi, tj)] = "dve"
            t_dve += cost_d
        else:
            assign[(wi, tj)] = "act"
            t_act += cost_a

    # huber shard sizes
    ch = -(-n // NCORES)  # per-core elems
    cht = -(-ch // P)  # free-dim cols of [128, cht] tile
    chp = cht * P

    nt = sum(w["T"] for w in windows)
    nacc = nt + 3
    # compact per-engine accumulator slots, in (window, tile) emission order
    slots = {}
    nd = na = 0
    for wi, w in enumerate(windows):
        for tj in range(w["T"]):
            if assign[(wi, tj)] == "dve":
                slots[(wi, tj)] = nd
                nd += 1
            else:
                slots[(wi, tj)] = na
                na += 1

    meta = dict(
        n=n,
        K=K,
        levels=levels,
        counts=counts.astype(np.int64),
        offs=offs,
        L=L,
        dead=dead,
        windows=windows,
        assign=assign,
        nt=nt,
        nacc=nacc,
        slots=slots,
        nd=nd,
        na=na,
        ch=ch,
        cht=cht,
        chp=chp,
        rt=int(cp_cores[0].shape[0]),
    )
    return meta, bcols, cp_cores, ps


def _shape_key(meta):
    wkey = tuple(
        (w["col0"], w["ncol"], w["T"]) for w in meta["windows"]
    )
    akey = tuple(sorted(meta["assign"].items()))
    return (meta["n"], meta["L"], meta["rt"], meta["cht"], wkey, akey)


@functools.lru_cache(maxsize=8)
def _build_program(key):
    """Raw Bass program (no TileContext): explicit per-engine streams and
    semaphores. Value-independent given the shape key."""
    n, L, rt, cht, wkey, akey = key
    assign = dict(akey)
    chp = cht * P
    nd = sum(1 for _, e in akey if e == "dve")
    na = sum(1 for _, e in akey if e == "act")

    nc = bacc.Bacc("TRN2", enable_partition_id=False)

    tcols = rt // P
    combw = 2 * tcols + 2 * cht  # [cp | cn | pred | targ], partition-major
    d_b = nc.dram_tensor("bcols", [max(L, 1)], mybir.dt.bfloat16, kind="ExternalInput")
    d_comb = nc.dram_tensor(
        "comb", [combw * P], mybir.dt.float32, kind="ExternalInput"
    )
    d_acc = nc.dram_tensor("acc", [P, max(nd, 1) + 1], mybir.dt.float32, kind="ExternalOutput")
    d_acc2 = nc.dram_tensor("acc2", [P, na + 3], mybir.dt.float32, kind="ExternalOutput")

    fp32 = mybir.dt.float32
    bf16 = mybir.dt.bfloat16
    Alu = mybir.AluOpType
    Act = mybir.ActivationFunctionType
    npf32 = fp32
    npbf16 = bf16

    maxncol = max((ncol for _, ncol, _ in wkey), default=1)
    nw = len(wkey)

    bt = nc.alloc_sbuf_tensor("bt", [P, max(L, 1)], npbf16)
    comb = nc.alloc_sbuf_tensor("comb_t", [P, combw], npf32)
    acc_d = nc.alloc_sbuf_tensor("acc_d", [P, max(nd, 1) + 1], npf32)
    acc_a = nc.alloc_sbuf_tensor("acc_a", [P, na + 3], npf32)
    n_scr_d = max(sum(1 for _, e in akey if e == "dve"), 1)
    n_scr_a = max(sum(1 for _, e in akey if e == "act"), 1)
    scr_ds = [
        nc.alloc_sbuf_tensor(f"scr_d{i}", [P, maxncol], npbf16)
        for i in range(n_scr_d)
    ]
    scr_as = [
        nc.alloc_sbuf_tensor(f"scr_a{i}", [P, maxncol], npbf16)
        for i in range(n_scr_a)
    ]
    hd = nc.alloc_sbuf_tensor("hd", [P, cht], npf32)
    hr1 = nc.alloc_sbuf_tensor("hr1", [P, cht], npf32)
    he = nc.alloc_sbuf_tensor("he", [P, cht], npf32)
    hr2 = nc.alloc_sbuf_tensor("hr2", [P, cht], npf32)
    hs = nc.alloc_sbuf_tensor("hs", [P, cht], npf32)
    hs1 = nc.alloc_sbuf_tensor("hs1", [P, cht], npf32)
    hs2 = nc.alloc_sbuf_tensor("hs2", [P, cht], npf32)

    s_comb = nc.alloc_semaphore("s_comb")
    s_cn = nc.alloc_semaphore("s_cn")
    s_w = [nc.alloc_semaphore(f"s_w{i}") for i in range(max(nw, 1))]
    s_w0s = [s_w[0]] + [nc.alloc_semaphore(f"s_w0c{j}") for j in range(1, 8)]
    s_hub = nc.alloc_semaphore("s_hub")
    s_dve = nc.alloc_semaphore("s_dve")
    s_act = nc.alloc_semaphore("s_act")
    s_dp = nc.alloc_semaphore("s_dp")
    s_ap = nc.alloc_semaphore("s_ap")
    s_out = nc.alloc_semaphore("s_out")

    pts = comb[:, 2 * tcols : 2 * tcols + cht]
    tts = comb[:, 2 * tcols + cht : 2 * tcols + 2 * cht]

    # per-engine tile worklists: (wi, col0, ncol, cidx)
    work_d, work_a = [], []
    colbase = 0
    for wi, (col0, ncol, T) in enumerate(wkey):
        for tj in range(T):
            item = (wi, col0, ncol, colbase + tj)
            (work_d if assign[(wi, tj)] == "dve" else work_a).append(item)
        colbase += T

    with nc.Block() as block:

        @block.sync
        def _(sync):
            sync.dma_start(
                out=comb[:, :], in_=d_comb[:].rearrange("(p t) -> p t", p=P)
            ).then_inc(s_comb, 16)
            if L > 0:
                col0, ncol, _T = wkey[0]
                kch = min(W0_CHUNKS, len(s_w0s)) if ncol >= 512 else 1
                bnds = [ncol * j // kch for j in range(kch + 1)]
                for j in range(kch):
                    o, c = bnds[j], bnds[j + 1] - bnds[j]
                    src = bass.AP(
                        tensor=d_b[:].tensor,
                        offset=col0 + o,
                        ap=[[0, P], [1, c]],
                    )
                    sync.dma_start(
                        out=bt[:, col0 + o : col0 + o + c], in_=src
                    ).then_inc(s_w0s[j], 16)
                for wi in range(1, nw):
                    wcol0, wncol, _T = wkey[wi]
                    src = bass.AP(
                        tensor=d_b[:].tensor, offset=wcol0, ap=[[0, P], [1, wncol]]
                    )
                    sync.dma_start(
                        out=bt[:, wcol0 : wcol0 + wncol], in_=src
                    ).then_inc(s_w[wi], 16)
            d_stage = 0  # staged out-DMA measured slower (ring overhead)
            a_stage = 0
            need = 32
            if d_stage > 0:
                sync.wait_ge(s_dp, 1)
                sync.dma_start(
                    out=d_acc[:, :d_stage], in_=acc_d[:, :d_stage]
                ).then_inc(s_out, 16)
                need += 16
            if a_stage > 0:
                sync.wait_ge(s_ap, 1)
                sync.dma_start(
                    out=d_acc2[:, :a_stage], in_=acc_a[:, :a_stage]
                ).then_inc(s_out, 16)
                need += 16
            sync.wait_ge(s_dve, 1)
            with nc.allow_non_contiguous_dma(reason="tiny tail accumulators"):
                sync.dma_start(
                    out=d_acc[:, d_stage:], in_=acc_d[:, d_stage:]
                ).then_inc(s_out, 16)
                sync.wait_ge(s_act, 1)
                sync.dma_start(
                    out=d_acc2[:, a_stage:], in_=acc_a[:, a_stage:]
                ).then_inc(s_out, 16)
            sync.wait_ge(s_out, need)



        @block.vector
        def _(vector):
            vector.wait_ge(s_comb, 16)
            # Huber elementwise prep first: fills the broadcast-DMA head and
            # unblocks ScalarE's squares early.
            vector.tensor_tensor(out=hd[:, :], in0=pts, in1=tts, op=Alu.subtract)
            vector.drain()
            vector.tensor_scalar(
                out=hr1[:, :], in0=hd[:, :], scalar1=1.0, scalar2=0.0,
                op0=Alu.subtract, op1=Alu.max,
            )
            vector.tensor_scalar(
                out=he[:, :], in0=hd[:, :], scalar1=-1.0, scalar2=1.0,
                op0=Alu.mult, op1=Alu.subtract,
            )
            vector.drain()
            vector.tensor_scalar(
                out=hr2[:, :], in0=he[:, :], scalar1=0.0, scalar2=None, op0=Alu.max,
            ).then_inc(s_hub, 1)
            w0_split = bool(work_d) and work_d[0][0] == 0 and wkey[0][1] >= 512
            kch = min(W0_CHUNKS, 8) if (L > 0 and wkey and wkey[0][1] >= 512) else 1
            last = None
            if nd == 0:
                last = vector.memset(acc_d[:, :], 0.0)
            elif not w0_split or kch == 1:
                last = vector.memset(acc_d[:, nd : nd + 1], 0.0)
            seen = set()
            for sl, (wi, col0, ncol, cidx) in enumerate(work_d):
                if sl == 0 and w0_split and kch > 1:
                    # window 0 arrives in kch chunk-DMAs; process the first
                    # tile chunk-by-chunk so compute starts as data lands.
                    # Chunks 1..kch-1 accumulate into the extra slot.
                    bnds = [ncol * j // kch for j in range(kch + 1)]
                    vector.wait_ge(s_w0s[0], 16)
                    vector.tensor_scalar(
                        out=scr_ds[sl][:, : bnds[1]],
                        in0=bt[:, col0 : col0 + bnds[1]],
                        scalar1=comb[:, tcols + cidx : tcols + cidx + 1],
                        scalar2=None,
                        op0=Alu.max,
                        op1=Alu.add,
                        accum_out=acc_d[:, sl : sl + 1],
                    )
                    if kch == 2:
                        vector.wait_ge(s_w0s[1], 16)
                        last = vector.tensor_scalar(
                            out=scr_ds[sl][:, bnds[1] : ncol],
                            in0=bt[:, col0 + bnds[1] : col0 + ncol],
                            scalar1=comb[:, tcols + cidx : tcols + cidx + 1],
                            scalar2=None,
                            op0=Alu.max,
                            op1=Alu.add,
                            accum_out=acc_d[:, nd : nd + 1],
                        )
                        seen.add(0)
                        continue
                    ex = nc.alloc_sbuf_tensor(f"exacc", [P, max(kch - 1, 1)], npf32)
                    for j in range(1, kch):
                        o, cw = bnds[j], bnds[j + 1] - bnds[j]
                        vector.wait_ge(s_w0s[j], 16)
                        last = vector.tensor_scalar(
                            out=scr_ds[sl][:, o : o + cw],
                            in0=bt[:, col0 + o : col0 + o + cw],
                            scalar1=comb[:, tcols + cidx : tcols + cidx + 1],
                            scalar2=None,
                            op0=Alu.max,
                            op1=Alu.add,
                            accum_out=ex[:, j - 1 : j],
                        )
                    # fold the chunk accums into the extra output slot
                    vector.drain()
                    last = vector.tensor_reduce(
                        out=acc_d[:, nd : nd + 1],
                        in_=ex[:, : kch - 1],
                        axis=mybir.AxisListType.X,
                        op=Alu.add,
                    )
                    seen.add(0)
                    continue
                if wi not in seen:
                    if wi == 0:
                        for j in range(kch):
                            vector.wait_ge(s_w0s[j], 16)
                    else:
                        vector.wait_ge(s_w[wi], 16)
                    seen.add(wi)
                last = vector.tensor_scalar(
                    out=scr_ds[sl][:, :ncol],
                    in0=bt[:, col0 : col0 + ncol],
                    scalar1=comb[:, tcols + cidx : tcols + cidx + 1],
                    scalar2=None,
                    op0=Alu.max,
                    op1=Alu.add,
                    accum_out=acc_d[:, sl : sl + 1],
                )
            last.then_inc(s_dve, 1)

        @block.scalar
        def _(act):
            act.wait_ge(s_hub, 1)
            act.activation(
                out=hs[:, :], in_=hd[:, :], func=Act.Square,
                accum_out=acc_a[:, na : na + 1],
            )
            act.activation(
                out=hs1[:, :], in_=hr1[:, :], func=Act.Square,
                accum_out=acc_a[:, na + 1 : na + 2],
            )
            last = act.activation(
                out=hs2[:, :], in_=hr2[:, :], func=Act.Square,
                accum_out=acc_a[:, na + 2 : na + 3],
            )
            kch_a = min(W0_CHUNKS, 8) if (L > 0 and wkey and wkey[0][1] >= 512) else 1
            seen = set()
            for sl, (wi, col0, ncol, cidx) in enumerate(work_a):
                if wi not in seen:
                    if wi == 0:
                        for j in range(kch_a):
                            act.wait_ge(s_w0s[j], 16)
                    else:
                        act.wait_ge(s_w[wi], 16)
                    seen.add(wi)
                last = act.activation(
                    out=scr_as[sl][:, :ncol],
                    in_=bt[:, col0 : col0 + ncol],
                    func=Act.Relu,
                    bias=comb[:, cidx : cidx + 1],
                    scale=1.0,
                    accum_out=acc_a[:, sl : sl + 1],
                )
            last.then_inc(s_act, 1)

    nc.finalize()
    return nc


def _make_inputs(meta, bcols, cp_cores, predictions, targets):
    n = meta["n"]
    chp = meta["chp"]
    cht = meta["cht"]
    L = meta["L"]
    rt = meta["rt"]
    in_maps = []
    b_in = np.ascontiguousarray(
        bcols if L > 0 else np.zeros(1, dtype=np.float32), dtype=ml_dtypes.bfloat16
    )
    for c in range(NCORES):
        pc = np.zeros(chp, dtype=np.float32)
        tc_ = np.zeros(chp, dtype=np.float32)
        lo = c * meta["ch"]
        hi = min((c + 1) * meta["ch"], n)
        if hi > lo:
            pc[: hi - lo] = predictions[lo:hi]
            tc_[: hi - lo] = targets[lo:hi]
        cp = cp_cores[c] if rt > 0 else np.zeros(0, dtype=np.float32)
        tcols = rt // P
        cols = []
        if tcols > 0:
            cols.append(cp.reshape(tcols, P).T)
            cols.append(-cp.reshape(tcols, P).T)
        cols.append(pc.reshape(cht, P).T)
        cols.append(tc_.reshape(cht, P).T)
        comb2d = np.concatenate(cols, axis=1).astype(np.float32)  # [128, combw]
        in_maps.append({"bcols": b_in, "comb": np.ascontiguousarray(comb2d.ravel())})
    return in_maps


def _gather(meta, cp_cores, results):
    """Combine per-core accumulators into the scalar loss (float64 host math)."""
    n = meta["n"]
    nt = meta["nt"]
    windows = meta["windows"]
    assign = meta["assign"]

    slots = meta["slots"]
    nd = meta["nd"]
    num = 0.0
    hub_a = hub_b = hub_c = 0.0
    for c in range(NCORES):
        acc = results[c]["acc"].astype(np.float64)
        acc2 = results[c]["acc2"].astype(np.float64)
        # hinge accumulators + DVE correction:  sum relu = accum + ncol * c_row
        colbase = 0
        for wi, w in enumerate(windows):
            for tj in range(w["T"]):
                sl = slots[(wi, tj)]
                if assign[(wi, tj)] == "dve":
                    num += acc[:, sl].sum()
                    if sl == 0:
                        num += acc[:, nd].sum()  # split-tile extra slot
                    rows = cp_cores[c][(colbase + tj) * P : (colbase + tj + 1) * P]
                    num += w["ncol"] * rows.astype(np.float64).sum()
                else:
                    num += acc2[:, sl].sum()
            colbase += w["T"]
        na = meta["na"]
        hub_a += acc2[:, na].sum()
        hub_b += acc2[:, na + 1].sum()
        hub_c += acc2[:, na + 2].sum()

    huber = 0.5 * (hub_a - hub_b - hub_c) / n

    counts = meta["counts"].astype(np.int64)
    csum = np.cumsum(counts)
    cnt = int(np.sum(counts[1:] * csum[:-1])) if len(counts) > 1 else 0
    if cnt > 0:
        ranking = num / float(np.float32(cnt))
    else:
        ranking = 0.0

    return np.float32(ALPHA * huber + BETA * ranking)


def _host_fallback(predictions, targets):
    """Safety net for input distributions the device plan is not built for
    (e.g. near-continuous targets). Exact O(n^2) evaluation, row-chunked."""
    p = predictions.astype(np.float64)
    t = targets.astype(np.float64)
    n = len(p)
    d = p - t
    ad = np.abs(d)
    huber = np.mean(np.where(ad < 1.0, 0.5 * d * d, ad - 0.5))
    num = 0.0
    cnt = 0
    step = 512
    for i0 in range(0, n, step):
        i1 = min(i0 + step, n)
        pd = p[i0:i1, None] - p[None, :]
        td = t[i0:i1, None] - t[None, :]
        sign = np.where(td > 0, 1.0, -1.0)
        idx = np.arange(n)
        mask = (td != 0) & (idx[i0:i1, None] < idx[None, :])
        hinge = np.maximum(0.0, 1.0 - sign * pd)
        num += hinge[mask].sum()
        cnt += int(mask.sum())
    ranking = num / float(np.float32(cnt)) if cnt > 0 else 0.0
    return np.float32(ALPHA * huber + BETA * ranking)


def kernel(predictions: np.ndarray, targets: np.ndarray) -> np.ndarray:
    predictions = np.asarray(predictions, dtype=np.float32)
    targets = np.asarray(targets, dtype=np.float32)

    if len(np.unique(targets)) > 16 or predictions.shape[0] < NCORES * P:
        return np.array(_host_fallback(predictions, targets), dtype=np.float32)

    meta, bcols, cp_cores, _ps = _plan(targets, predictions)
    nc = _build_program(_shape_key(meta))
    in_maps = _make_inputs(meta, bcols, cp_cores, predictions, targets)
    res = run_bass_kernel_spmd(nc, in_maps, list(range(NCORES)))
    return np.array(_gather(meta, cp_cores, res.results), dtype=np.float32)



# revision 6
# speedup vs baseline: 1.0113x; 1.0113x over previous
"""Trainium2 Bass kernel for EnhancedGradedLoss (Huber + pairwise hinge ranking).

Algorithm (see reference): loss = 0.7 * SmoothL1(p, t) + 0.3 * ranking, where
ranking averages relu(1 - sign(t_i - t_j) * (p_i - p_j)) over i<j pairs with
t_i != t_j.

Device strategy (8 NeuronCores, SPMD), v2:
  * Host sorts items by grade. Cross-grade pairs decompose via a binary split
    of the grade set: pairs(lo-set x hi-set) form one rectangular "group"
    (rows = one side, cols = the other), recursing into each half. A group may
    be FLIPPED (rows = the lower-grade set, cols = negated upper-grade preds)
    when that shards into fewer [128 x ncol] tiles. For the 4-grade case this
    yields 8 row-tiles/core covering all 24.6M cross pairs with ~0.5% padding
    waste (vs 13 tiles for the per-grade window form).
  * The sorted prediction segments ("B") are broadcast to all 128 SBUF
    partitions per core via stride-0 DMAs (bf16), chunked so compute streams
    behind the DMA. The first chunk + the per-row-constant table are issued
    BEFORE the block entry barrier, overlapping the ~1.3us DMA dispatch with
    the framework preamble.
  * Three engines compute the hinge sum concurrently:
      - DVE:  tensor_scalar(max, scalar=-c, accum_out)  @ 4x bf16
      - ACT:  activation(Relu, bias=c, accum_out)
      - Pool: tensor_scalar(max) + tensor_reduce(XYZWC)  (accum_out does not
        compile on GPSIMD, so a full-tile reduce supplies the sum)
    using sum_j relu(B_j + c) = sum_j max(B_j, -c) + ncol * c (host adds the
    closed-form correction in float64).
  * Huber runs entirely on Pool (prep + squares + one fused reduce), freeing
    DVE/ACT for hinge throughput.
  * A single merged output DMA returns all accumulators ([128, S] f32).
  * Raw Bass program (no Tile framework); per-core differences are carried
    entirely by input data, so one SPMD program serves all 8 cores.
"""

import functools
import sys

import ml_dtypes
import numpy as np

sys.path.insert(0, "/opt/trn_rl_repo")

import concourse.bacc as bacc
import concourse.bass as bass
from concourse import mybir
from concourse.bass_utils import run_bass_kernel_spmd

ALPHA = 0.7
BETA = 0.3
NCORES = 8
P = 128

# --- cost/latency model constants (mirrors bass_rust cost model, TRN2) -----
_HWDGE = 625.0
_DGE = 650.0
_SEM_DMA = 900.0
_DVE_COL = 1e9 / 0.96e9 * 0.25   # 4x bf16
_DVE_FIX = 61.0                  # SBUF access init (58*2*cyc/2)
_ACT_COL = 1e9 / 1.2e9
_ACT_FIX = 372.0                 # SBUF init half + accum-read 187
_POOL_COL = 2.0 * (1e9 / 1.2e9) / 0.6   # ts-max + reduce passes
_POOL_FIX = 2.0 * 95.0 + 40.0


def _plan(targets_f, predictions_f):
    """Host-side planning: sort by grade, pair-group decomposition, broadcast
    layout, DMA chunking, and 3-engine work assignment."""
    n = targets_f.shape[0]
    order = np.argsort(targets_f, kind="stable")
    ts = targets_f[order]
    ps = predictions_f[order].astype(np.float32)

    levels, counts = np.unique(ts, return_counts=True)
    K = len(levels)
    offs = np.concatenate([[0], np.cumsum(counts)]).astype(np.int64)

    pmax = float(np.max(np.abs(ps))) if n else 0.0
    dead = -float(np.float32(np.ceil(pmax) + 2.0))

    # --- pair groups via binary grade split, with per-group flip choice ----
    def tiles_of(m):
        q = -(-m // NCORES)
        return -(-q // P)

    groups = []  # dicts: rows (lo,hi sorted-pos), sign, cols (lo,hi), flip

    def rec(a, b):
        if b - a < 2:
            return
        mid = (a + b) // 2
        m_un = int(offs[b] - offs[mid])     # rows = upper set, c = 1 - p
        m_fl = int(offs[mid] - offs[a])     # rows = lower set, c = 1 + p
        ncol_un = int(offs[mid] - offs[a])  # cols = +p of lower set
        ncol_fl = int(offs[b] - offs[mid])  # cols = -p of upper set
        cost_un = tiles_of(m_un) * ncol_un
        cost_fl = tiles_of(m_fl) * ncol_fl
        if m_un and ncol_un:
            if cost_fl < cost_un:
                groups.append(
                    dict(rlo=int(offs[a]), rhi=int(offs[mid]), flip=True,
                         clo=int(offs[mid]), chi=int(offs[b]))
                )
            else:
                groups.append(
                    dict(rlo=int(offs[mid]), rhi=int(offs[b]), flip=False,
                         clo=int(offs[a]), chi=int(offs[mid]))
                )
        rec(a, mid)
        rec(mid, b)

    rec(0, K)

    # --- broadcast layout: place (sign, lo, hi) column sets ---------------
    # +p segments first (sorted-prefix ranges nest), then -p segments.
    placed = []  # (sign, lo, hi, layout_start)
    cursor = 0
    for g in sorted(groups, key=lambda g: (g["flip"], -(g["chi"] - g["clo"]))):
        sgn = -1 if g["flip"] else 1
        hit = None
        for (s2, lo2, hi2, st2) in placed:
            if s2 == sgn and lo2 <= g["clo"] and g["chi"] <= hi2:
                hit = st2 + (g["clo"] - lo2)
                break
        if hit is None:
            hit = cursor
            placed.append((sgn, g["clo"], g["chi"], cursor))
            cursor += g["chi"] - g["clo"]
        g["bc0"] = int(hit)  # layout col of this group's first B column
    L = cursor

    bcols = np.zeros(max(L, 1), dtype=np.float32)
    for (sgn, lo, hi, st) in placed:
        bcols[st : st + (hi - lo)] = sgn * ps[lo:hi]

    # --- per-group sharding ------------------------------------------------
    for g in groups:
        m = g["rhi"] - g["rlo"]
        g["m"] = m
        g["q"] = -(-m // NCORES)
        g["T"] = -(-g["q"] // P)
        g["ncol"] = g["chi"] - g["clo"]

    G = sum(g["T"] for g in groups)  # total row-tiles per core

    # per-core row-constant arrays (group-major, each padded to T*128).
    # cp = +c (ACT bias / host correction), cn = -c (DVE/Pool max scalar).
    cp_cores = []
    for ci in range(NCORES):
        parts = []
        for g in groups:
            r0 = g["rlo"] + ci * g["q"]
            r1 = min(g["rlo"] + min((ci + 1) * g["q"], g["m"]), g["rhi"])
            r0 = min(r0, r1)
            pv = ps[r0:r1]
            c = (np.float32(1.0) + pv) if g["flip"] else (np.float32(1.0) - pv)
            padded = np.full(g["T"] * P, dead, dtype=np.float32)
            padded[: len(c)] = c.astype(np.float32)
            parts.append(padded)
        cp_cores.append(
            np.concatenate(parts) if parts else np.zeros(0, dtype=np.float32)
        )

    # --- DMA chunk plan over the broadcast layout -------------------------
    # Cut at group-column boundaries; subdivide long runs to ~1030 cols.
    cuts = {0, L}
    for g in groups:
        cuts.add(g["bc0"])
        cuts.add(g["bc0"] + g["ncol"])
    cuts = sorted(cuts)
    chunks = []  # (lo, hi) layout ranges
    for lo, hi in zip(cuts[:-1], cuts[1:]):
        span = hi - lo
        if span <= 0:
            continue
        k = max(1, -(-span // 1040))
        bnds = [lo + span * j // k for j in range(k + 1)]
        for j in range(k):
            chunks.append((bnds[j], bnds[j + 1]))
    NCH = len(chunks)

    # DMA issue order: by total work units unlocked (multiplicity * span),
    # descending multiplicity == ascending layout order for our groups, so
    # natural order is fine; keep layout order.
    # Arrival model:
    t0 = 50.0
    n_preblock = 2  # chunk0 + comb before the entry barrier
    hwdge_end = []
    dma_end = []
    sems = []
    prev_h = t0
    prev_d = 0.0
    # order of DMAs: chunk0, comb, chunk1.., (output handled separately)
    sizes = []
    for i in range(NCH):
        lo, hi = chunks[i]
        sizes.append((hi - lo) * 2 * P)  # bf16 dst bytes
    ch = -(-n // NCORES)
    cht = -(-ch // P)
    combw = 2 * G + 2 * cht
    dma_order = [("chunk", 0), ("comb", -1)] + [("chunk", i) for i in range(1, NCH)]
    arrival = [0.0] * NCH
    comb_arrival = 0.0
    for (kind, idx) in dma_order:
        h_end = prev_h + _HWDGE
        prev_h = h_end
        if kind == "comb":
            nbytes = combw * 4 * P
            elem = combw * 4
        else:
            nbytes = sizes[idx]
            elem = (chunks[idx][1] - chunks[idx][0]) * 2
        mult = 2.0 if elem < 512 else 1.0
        tr = max(nbytes * mult / 360.0, 7.0)
        d_start = max(h_end + _DGE, prev_d)
        d_end = d_start + tr
        prev_d = d_end
        sem_t = d_end + _SEM_DMA + 30.0
        if kind == "comb":
            comb_arrival = sem_t
        else:
            arrival[idx] = sem_t

    barrier_release = t0 + n_preblock * 650.0 + 750.0

    # --- work items and greedy 3-engine scheduling ------------------------
    # item: (gi, tile, chunk_idx, c0_local, ncol)  c0_local = col offset in
    # the group's column range; engines read bt[:, bc0 + c0_local : ...].
    items = []
    for gi, g in enumerate(groups):
        glo, ghi = g["bc0"], g["bc0"] + g["ncol"]
        for cidx, (lo, hi) in enumerate(chunks):
            s, e = max(lo, glo), min(hi, ghi)
            if s >= e:
                continue
            for tj in range(g["T"]):
                items.append(dict(gi=gi, tj=tj, cidx=cidx, c0=s - glo, nc=e - s))
    items.sort(key=lambda it: (arrival[it["cidx"]], it["gi"], it["tj"]))

    eng_free = {
        "dve": max(barrier_release, 0.0),
        "act": max(barrier_release, 0.0),
        "pool": max(barrier_release, comb_arrival) + 1150.0,  # huber first
    }

    def cost(eng, nc):
        if eng == "dve":
            return nc * _DVE_COL + _DVE_FIX
        if eng == "act":
            return nc * _ACT_COL + _ACT_FIX
        return nc * _POOL_COL + _POOL_FIX

    assign = []  # (engine, merged-item)
    pend = list(items)
    i = 0
    while i < len(pend):
        it = pend[i]
        best = None
        for eng in ("dve", "act", "pool"):
            st = max(eng_free[eng], arrival[it["cidx"]], comb_arrival)
            fin = st + cost(eng, it["nc"])
            if best is None or fin < best[0]:
                best = (fin, eng, st)
        fin, eng, st = best
        # merge following chunks of the same (gi, tj) if already arrived
        merged = dict(it)
        j = i + 1
        while j < len(pend):
            nx = pend[j]
            if (
                nx["gi"] == merged["gi"]
                and nx["tj"] == merged["tj"]
                and nx["c0"] == merged["c0"] + merged["nc"]
                and arrival[nx["cidx"]] <= st
            ):
                merged["nc"] += nx["nc"]
                merged["cidx"] = max(merged["cidx"], nx["cidx"])
                fin = st + cost(eng, merged["nc"])
                pend.pop(j)
            else:
                break
        assign.append((eng, merged))
        eng_free[eng] = fin
        i += 1

    # per-engine ordered worklists with accumulator slots
    work = {"dve": [], "act": [], "pool": []}
    for eng, it in assign:
        work[eng].append(it)
    for eng in work:
        work[eng].sort(key=lambda it: (arrival[it["cidx"]], it["gi"], it["tj"]))
    nd = len(work["dve"])
    na = len(work["act"])
    npo = len(work["pool"])
    S = nd + na + npo + 1  # + huber scalar slot

    meta = dict(
        n=n,
        K=K,
        levels=levels,
        counts=counts.astype(np.int64),
        offs=offs,
        L=L,
        dead=dead,
        groups=groups,
        chunks=chunks,
        work=work,
        nd=nd,
        na=na,
        npo=npo,
        S=S,
        G=G,
        ch=ch,
        cht=cht,
        chp=cht * P,
        rt=int(cp_cores[0].shape[0]),
    )
    return meta, bcols, cp_cores, ps


def _shape_key(meta):
    gkey = tuple(
        (g["rlo"], g["rhi"], g["flip"], g["clo"], g["chi"], g["bc0"], g["T"])
        for g in meta["groups"]
    )
    ckey = tuple(meta["chunks"])
    wkey = tuple(
        (eng, tuple((it["gi"], it["tj"], it["cidx"], it["c0"], it["nc"])
                    for it in meta["work"][eng]))
        for eng in ("dve", "act", "pool")
    )
    return (meta["n"], meta["L"], meta["rt"], meta["cht"], gkey, ckey, wkey)


@functools.lru_cache(maxsize=8)
def _build_program(key):
    """Raw Bass program: explicit per-engine streams and semaphores."""
    n, L, rt, cht, gkey, ckey, wkey = key
    groups = [
        dict(rlo=a, rhi=b, flip=f, clo=c, chi=d, bc0=e, T=t, ncol=d - c)
        for (a, b, f, c, d, e, t) in gkey
    ]
    chunks = list(ckey)
    work = {eng: [dict(gi=gi, tj=tj, cidx=ci, c0=c0, nc=nc)
                  for (gi, tj, ci, c0, nc) in wl]
            for (eng, wl) in wkey}
    nd, na, npo = len(work["dve"]), len(work["act"]), len(work["pool"])
    S = nd + na + npo + 1
    G = sum(g["T"] for g in groups)
    combw = 2 * G + 2 * cht
    NCH = len(chunks)

    # comb column index of a (group, tile)'s row-constant
    tbase = {}
    b = 0
    for gi, g in enumerate(groups):
        for tj in range(g["T"]):
            tbase[(gi, tj)] = b
            b += 1

    nc = bacc.Bacc("TRN2", enable_partition_id=False)

    fp32 = mybir.dt.float32
    bf16 = mybir.dt.bfloat16
    Alu = mybir.AluOpType
    Act = mybir.ActivationFunctionType

    d_b = nc.dram_tensor("bcols", [max(L, 1)], bf16, kind="ExternalInput")
    d_comb = nc.dram_tensor("comb", [combw * P], fp32, kind="ExternalInput")
    d_acc = nc.dram_tensor("acc", [P, S], fp32, kind="ExternalOutput")

    bt = nc.alloc_sbuf_tensor("bt", [P, max(L, 1)], bf16)
    comb = nc.alloc_sbuf_tensor("comb_t", [P, combw], fp32)
    acc = nc.alloc_sbuf_tensor("acc_t", [P, S], fp32)

    max_d = max([it["nc"] for it in work["dve"]], default=1)
    max_a = max([it["nc"] for it in work["act"]], default=1)
    max_p = max([it["nc"] for it in work["pool"]], default=1)
    scr_d = [nc.alloc_sbuf_tensor(f"scr_d{i}", [P, max_d], bf16) for i in range(2)]
    scr_a = [nc.alloc_sbuf_tensor(f"scr_a{i}", [P, max_a], bf16) for i in range(2)]
    scr_p = [nc.alloc_sbuf_tensor(f"scr_p{i}", [P, max_p], fp32) for i in range(2)]

    # huber scratch (Pool): hd, r1, he, r2, and three squares side by side
    hd = nc.alloc_sbuf_tensor("hd", [P, cht], fp32)
    hr1 = nc.alloc_sbuf_tensor("hr1", [P, cht], fp32)
    he = nc.alloc_sbuf_tensor("he", [P, cht], fp32)
    hr2 = nc.alloc_sbuf_tensor("hr2", [P, cht], fp32)
    hsq = nc.alloc_sbuf_tensor("hsq", [P, cht], fp32)
    hs1 = nc.alloc_sbuf_tensor("hs1", [P, cht], fp32)
    hs2 = nc.alloc_sbuf_tensor("hs2", [P, cht], fp32)

    s_ch = [nc.alloc_semaphore(f"s_ch{i}") for i in range(max(NCH, 1))]
    s_comb = nc.alloc_semaphore("s_comb")
    s_dve = nc.alloc_semaphore("s_dve")
    s_act = nc.alloc_semaphore("s_act")
    s_pool = nc.alloc_semaphore("s_pool")
    s_out = nc.alloc_semaphore("s_out")

    cn_col = lambda gi, tj: comb[:, tbase[(gi, tj)] : tbase[(gi, tj)] + 1]
    cp_col = lambda gi, tj: comb[:, G + tbase[(gi, tj)] : G + tbase[(gi, tj)] + 1]
    pts = comb[:, 2 * G : 2 * G + cht]
    tts = comb[:, 2 * G + cht : 2 * G + 2 * cht]

    def bcast_dma(sync_eng, cidx):
        lo, hi = chunks[cidx]
        src = bass.AP(tensor=d_b[:].tensor, offset=lo, ap=[[0, P], [1, hi - lo]])
        sync_eng.dma_start(out=bt[:, lo:hi], in_=src).then_inc(s_ch[cidx], 16)

    # --- pre-barrier DMAs: first chunk + comb ------------------------------
    if L > 0 and NCH > 0:
        bcast_dma(nc.sync, 0)
    nc.sync.dma_start(
        out=comb[:, :], in_=d_comb[:].rearrange("(p t) -> p t", p=P)
    ).then_inc(s_comb, 16)

    def emit_stream(eng, wl, scr, mk_inst, done_sem):
        waited = set()
        last = None
        for k, it in enumerate(wl):
            # wait for every chunk this (possibly merged) item covers
            g = groups[it["gi"]]
            lo = g["bc0"] + it["c0"]
            hi = lo + it["nc"]
            for ci, (clo, chi) in enumerate(chunks):
                if clo < hi and lo < chi and ci not in waited:
                    eng.wait_ge(s_ch[ci], 16)
                    waited.add(ci)
            if k == 0:
                eng.wait_ge(s_comb, 16)
            last = mk_inst(eng, k, it, scr[k % 2], lo, hi)
        if last is not None:
            last.then_inc(done_sem, 1)
        return last

    with nc.Block() as block:

        @block.sync
        def _(sync):
            for ci in range(1, NCH):
                bcast_dma(sync, ci)
            sync.wait_ge(s_dve, 1)
            sync.wait_ge(s_act, 1)
            sync.wait_ge(s_pool, 1)
            with nc.allow_non_contiguous_dma(reason="small accumulator tile"):
                sync.dma_start(out=d_acc[:, :], in_=acc[:, :]).then_inc(s_out, 16)
            sync.wait_ge(s_out, 16)

        @block.vector
        def _(vector):
            def mk(eng, k, it, scr, lo, hi):
                return eng.tensor_scalar(
                    out=scr[:, : it["nc"]],
                    in0=bt[:, lo:hi],
                    scalar1=cn_col(it["gi"], it["tj"]),
                    scalar2=None,
                    op0=Alu.max,
                    op1=Alu.add,
                    accum_out=acc[:, k : k + 1],
                )
            if work["dve"]:
                emit_stream(vector, work["dve"], scr_d, mk, s_dve)
            else:
                vector.wait_ge(s_comb, 16)
                vector.tensor_scalar(
                    out=scr_d[0][:, 0:1], in0=comb[:, 0:1], scalar1=0.0,
                    scalar2=None, op0=Alu.add,
                ).then_inc(s_dve, 1)

        @block.scalar
        def _(act):
            def mk(eng, k, it, scr, lo, hi):
                return eng.activation(
                    out=scr[:, : it["nc"]],
                    in_=bt[:, lo:hi],
                    func=Act.Relu,
                    bias=cp_col(it["gi"], it["tj"]),
                    scale=1.0,
                    accum_out=acc[:, nd + k : nd + k + 1],
                )
            if work["act"]:
                emit_stream(act, work["act"], scr_a, mk, s_act)
            else:
                act.wait_ge(s_comb, 16)
                act.activation(
                    out=scr_a[0][:, 0:1], in_=comb[:, 0:1], func=Act.Relu,
                    bias=0.0, scale=1.0,
                ).then_inc(s_act, 1)

        @block.gpsimd
        def _(pool):
            # Huber first: needs only comb. hd = p - t; r1 = max(d-1, 0);
            # he = -d - 1; r2 = max(he, 0); slot = sum(d^2 - r1^2 - r2^2).
            pool.memset(acc[:, nd + na : S], 0.0)
            pool.wait_ge(s_comb, 16)
            pool.tensor_tensor(out=hd[:, :], in0=pts, in1=tts, op=Alu.subtract)
            pool.tensor_scalar(
                out=hr1[:, :], in0=hd[:, :], scalar1=1.0, scalar2=0.0,
                op0=Alu.subtract, op1=Alu.max,
            )
            pool.tensor_scalar(
                out=he[:, :], in0=hd[:, :], scalar1=-1.0, scalar2=1.0,
                op0=Alu.mult, op1=Alu.subtract,
            )
            pool.tensor_scalar(
                out=hr2[:, :], in0=he[:, :], scalar1=0.0, scalar2=None,
                op0=Alu.max,
            )
            pool.tensor_tensor(out=hsq[:, :], in0=hd[:, :], in1=hd[:, :], op=Alu.mult)
            pool.tensor_tensor(out=hs1[:, :], in0=hr1[:, :], in1=hr1[:, :], op=Alu.mult)
            pool.tensor_tensor(out=hs2[:, :], in0=hr2[:, :], in1=hr2[:, :], op=Alu.mult)
            pool.tensor_tensor(out=hsq[:, :], in0=hsq[:, :], in1=hs1[:, :], op=Alu.subtract)
            pool.tensor_tensor(out=hsq[:, :], in0=hsq[:, :], in1=hs2[:, :], op=Alu.subtract)
            hub = pool.tensor_reduce(
                out=acc[0:1, S - 1 : S], in_=hsq[:, :],
                axis=mybir.AxisListType.XYZWC, op=Alu.add,
            )
            if work["pool"]:
                waited = set()
                last = None
                for k, it in enumerate(work["pool"]):
                    g = groups[it["gi"]]
                    lo = g["bc0"] + it["c0"]
                    hi = lo + it["nc"]
                    for ci, (clo, chi) in enumerate(chunks):
                        if clo < hi and lo < chi and ci not in waited:
                            pool.wait_ge(s_ch[ci], 16)
                            waited.add(ci)
                    scr = scr_p[k % 2]
                    pool.tensor_scalar(
                        out=scr[:, : it["nc"]], in0=bt[:, lo:hi],
                        scalar1=cn_col(it["gi"], it["tj"]), scalar2=None,
                        op0=Alu.max,
                    )
                    last = pool.tensor_reduce(
                        out=acc[0 : 1, nd + na + k : nd + na + k + 1],
                        in_=scr[:, : it["nc"]],
                        axis=mybir.AxisListType.XYZWC, op=Alu.add,
                    )
                last.then_inc(s_pool, 1)
            else:
                hub.then_inc(s_pool, 1)

    nc.finalize()
    return nc


def _make_inputs(meta, bcols, cp_cores, predictions, targets):
    n = meta["n"]
    cht = meta["cht"]
    chp = meta["chp"]
    L = meta["L"]
    G = meta["G"]
    groups = meta["groups"]
    in_maps = []
    b_in = np.ascontiguousarray(
        bcols if L > 0 else np.zeros(1, dtype=np.float32), dtype=ml_dtypes.bfloat16
    )
    for ci in range(NCORES):
        pc = np.zeros(chp, dtype=np.float32)
        tc_ = np.zeros(chp, dtype=np.float32)
        lo = ci * meta["ch"]
        hi = min((ci + 1) * meta["ch"], n)
        if hi > lo:
            pc[: hi - lo] = predictions[lo:hi]
            tc_[: hi - lo] = targets[lo:hi]
        cp = cp_cores[ci]
        cols = []
        if G > 0:
            cols.append(-cp.reshape(G, P).T)  # cn = -c
            cols.append(cp.reshape(G, P).T)   # cp = +c
        cols.append(pc.reshape(cht, P).T)
        cols.append(tc_.reshape(cht, P).T)
        comb2d = np.concatenate(cols, axis=1).astype(np.float32)  # [128, combw]
        in_maps.append({"bcols": b_in, "comb": np.ascontiguousarray(comb2d.ravel())})
    return in_maps


def _gather(meta, cp_cores, results):
    """Combine per-core accumulators into the scalar loss (float64 host math)."""
    n = meta["n"]
    groups = meta["groups"]
    work = meta["work"]
    nd = meta["nd"]
    na = meta["na"]
    S = meta["S"]

    # comb tile base index per (gi, tj) to locate cp rows in cp_cores
    tbase = {}
    b = 0
    for gi, g in enumerate(groups):
        for tj in range(g["T"]):
            tbase[(gi, tj)] = b
            b += 1

    num = 0.0
    hub = 0.0
    for ci in range(NCORES):
        acc = results[ci]["acc"].astype(np.float64)
        cpv = cp_cores[ci].astype(np.float64)
        for k, it in enumerate(work["dve"]):
            tb = tbase[(it["gi"], it["tj"])]
            rows = cpv[tb * P : (tb + 1) * P]
            num += acc[:, k].sum() + it["nc"] * rows.sum()
        for k, it in enumerate(work["act"]):
            num += acc[:, nd + k].sum()
        for k, it in enumerate(work["pool"]):
            tb = tbase[(it["gi"], it["tj"])]
            rows = cpv[tb * P : (tb + 1) * P]
            num += acc[0, nd + na + k] + it["nc"] * rows.sum()
        hub += acc[0, S - 1]

    huber = 0.5 * hub / n

    counts = meta["counts"].astype(np.int64)
    csum = np.cumsum(counts)
    cnt = int(np.sum(counts[1:] * csum[:-1])) if len(counts) > 1 else 0
    ranking = num / float(np.float32(cnt)) if cnt > 0 else 0.0

    return np.float32(ALPHA * huber + BETA * ranking)


def _host_fallback(predictions, targets):
    """Safety net for input distributions the device plan is not built for
    (e.g. near-continuous targets). Exact O(n^2) evaluation, row-chunked."""
    p = predictions.astype(np.float64)
    t = targets.astype(np.float64)
    n = len(p)
    d = p - t
    ad = np.abs(d)
    huber = np.mean(np.where(ad < 1.0, 0.5 * d * d, ad - 0.5))
    num = 0.0
    cnt = 0
    step = 512
    for i0 in range(0, n, step):
        i1 = min(i0 + step, n)
        pd = p[i0:i1, None] - p[None, :]
        td = t[i0:i1, None] - t[None, :]
        sign = np.where(td > 0, 1.0, -1.0)
        idx = np.arange(n)
        mask = (td != 0) & (idx[i0:i1, None] < idx[None, :])
        hinge = np.maximum(0.0, 1.0 - sign * pd)
        num += hinge[mask].sum()
        cnt += int(mask.sum())
    ranking = num / float(np.float32(cnt)) if cnt > 0 else 0.0
    return np.float32(ALPHA * huber + BETA * ranking)


def kernel(predictions: np.ndarray, targets: np.ndarray) -> np.ndarray:
    predictions = np.asarray(predictions, dtype=np.float32)
    targets = np.asarray(targets, dtype=np.float32)

    if (
        len(np.unique(targets)) > 16
        or len(np.unique(targets)) < 2
        or predictions.shape[0] < NCORES * P
    ):
        return np.array(_host_fallback(predictions, targets), dtype=np.float32)

    meta, bcols, cp_cores, _ps = _plan(targets, predictions)
    nc = _build_program(_shape_key(meta))
    in_maps = _make_inputs(meta, bcols, cp_cores, predictions, targets)
    res = run_bass_kernel_spmd(nc, in_maps, list(range(NCORES)))
    return np.array(_gather(meta, cp_cores, res.results), dtype=np.float32)


# revision 14
# speedup vs baseline: 1.0710x; 1.0591x over previous
"""Trainium2 Bass kernel for EnhancedGradedLoss (Huber + pairwise hinge ranking).

Algorithm (see reference): loss = 0.7 * SmoothL1(p, t) + 0.3 * ranking, where
ranking averages relu(1 - sign(t_i - t_j) * (p_i - p_j)) over i<j pairs with
t_i != t_j.

Device strategy (8 NeuronCores, SPMD), v3:
  * Host sorts items by grade. Cross-grade pairs decompose via a binary split
    of the grade set: pairs(lo-set x hi-set) form one rectangular "group"
    (rows x cols), recursing into each half. A group is FLIPPED (rows = the
    lower-grade set, cols = negated upper-grade preds) when that shards into
    fewer [128 x ncol] tiles. For 4 grades this covers all 24.6M cross pairs
    in 8 row-tiles/core with ~0.5% padding waste.
  * All device data is bf16. The first DMA is a "hot" [128, combw+c0] image:
    per-row constants (-c | +c), the huber pred/targ shard, and the first
    broadcast chunk - so every engine can start as soon as one DMA lands
    (~3.3us: barrier 0.64 + HWDGE 0.63 + DGE 0.65 + transfer 0.4 + sem 0.93).
    It is issued BEFORE the block entry barrier, as is the second chunk.
    Remaining chunks stream via stride-0 broadcast DMAs (dst bytes / 360GB/s
    is the modeled wall: ~4.4us of DMA-pipe time for the 1.5MB broadcast).
  * Three engines consume the hinge tiles concurrently:
      - DVE:  tensor_scalar(max, scalar=-c, accum_out)       ~0.26 ns/col
      - ACT:  activation(Relu, bias=c, accum_out)            ~0.83 ns/col
      - Pool: tensor_scalar(max) + tensor_reduce(XYZWC)      ~2.8  ns/col
        (accum_out does not compile on GPSIMD; a full-tile reduce sums)
    using sum_j relu(B_j + c) = sum_j max(B_j, -c) + ncol * c, corrected on
    host in float64. Work items are assigned by a waterfill scheduler
    (slowest engine that still meets the modeled makespan target) honoring
    per-chunk DMA arrival times, and adjacent chunks merge into longer
    instructions once the stream runs behind the engines.
  * Huber runs entirely on Pool (prep + squares + one fused reduce).
  * ACT opens with a dummy no-wait activation so the 1.28us activation-table
    load hoists into the DMA head instead of serializing with real work.
  * One merged output DMA returns all accumulators ([128, S] f32).
"""

import functools
import sys

import ml_dtypes
import numpy as np

sys.path.insert(0, "/opt/trn_rl_repo")

import concourse.bacc as bacc
import concourse.bass as bass
from concourse import mybir
from concourse.bass_utils import run_bass_kernel_spmd

ALPHA = 0.7
BETA = 0.3
NCORES = 8
P = 128

# --- cost/latency model constants (mirrors bass_rust cost model, TRN2) -----
_HWDGE = 625.0
_DGE = 650.0
_SEM_DMA = 930.0
_T0 = 641.0                      # first HWDGE slot (after init pseudo-barrier)
_DVE_COL = 1e9 / 0.96e9 * 0.25   # 4x bf16
_DVE_FIX = 61.0
_ACT_COL = 1e9 / 1.2e9
_ACT_FIX = 372.0                 # SBUF init half + accum-read 187
_POOL_COL = 2.0 * (1e9 / 1.2e9) / 0.6   # ts-max + reduce passes
_POOL_FIX = 2.0 * 95.0 + 50.0
_HUBER_POOL = 1200.0             # memset + 9 ops + reduce
_C0 = 512                        # broadcast cols riding the hot DMA


def _cost(eng, nc_):
    if eng == "dve":
        return nc_ * _DVE_COL + _DVE_FIX
    if eng == "act":
        return nc_ * _ACT_COL + _ACT_FIX
    return nc_ * _POOL_COL + _POOL_FIX


def _plan(targets_f, predictions_f):
    """Host-side planning: sort by grade, pair-group decomposition, broadcast
    layout, DMA chunking, and 3-engine work assignment."""
    n = targets_f.shape[0]
    order = np.argsort(targets_f, kind="stable")
    ts = targets_f[order]
    ps = predictions_f[order].astype(np.float32)

    levels, counts = np.unique(ts, return_counts=True)
    K = len(levels)
    offs = np.concatenate([[0], np.cumsum(counts)]).astype(np.int64)

    pmax = float(np.max(np.abs(ps))) if n else 0.0
    dead = -float(np.float32(np.ceil(pmax) + 2.0))

    # --- pair groups via binary grade split, with per-group flip choice ----
    def tiles_of(m):
        q = -(-m // NCORES)
        return -(-q // P)

    groups = []

    def rec(a, b):
        if b - a < 2:
            return
        mid = (a + b) // 2
        m_un = int(offs[b] - offs[mid])
        m_fl = int(offs[mid] - offs[a])
        ncol_un = int(offs[mid] - offs[a])
        ncol_fl = int(offs[b] - offs[mid])
        if m_un and ncol_un:
            if tiles_of(m_fl) * ncol_fl < tiles_of(m_un) * ncol_un:
                groups.append(
                    dict(rlo=int(offs[a]), rhi=int(offs[mid]), flip=True,
                         clo=int(offs[mid]), chi=int(offs[b]))
                )
            else:
                groups.append(
                    dict(rlo=int(offs[mid]), rhi=int(offs[b]), flip=False,
                         clo=int(offs[a]), chi=int(offs[mid]))
                )
        rec(a, mid)
        rec(mid, b)

    rec(0, K)

    # --- broadcast layout --------------------------------------------------
    placed = []
    cursor = 0
    for g in sorted(groups, key=lambda g: (g["flip"], -(g["chi"] - g["clo"]))):
        sgn = -1 if g["flip"] else 1
        hit = None
        for (s2, lo2, hi2, st2) in placed:
            if s2 == sgn and lo2 <= g["clo"] and g["chi"] <= hi2:
                hit = st2 + (g["clo"] - lo2)
                break
        if hit is None:
            hit = cursor
            placed.append((sgn, g["clo"], g["chi"], cursor))
            cursor += g["chi"] - g["clo"]
        g["bc0"] = int(hit)
    L = cursor

    bcols = np.zeros(max(L, 1), dtype=np.float32)
    for (sgn, lo, hi, st) in placed:
        bcols[st : st + (hi - lo)] = sgn * ps[lo:hi]

    for g in groups:
        m = g["rhi"] - g["rlo"]
        g["m"] = m
        g["q"] = -(-m // NCORES)
        g["T"] = -(-g["q"] // P)
        g["ncol"] = g["chi"] - g["clo"]

    G = sum(g["T"] for g in groups)
    ch = -(-n // NCORES)
    cht = -(-ch // P)
    combw = 2 * G + 2 * cht

    # per-core +c row constants, bf16-rounded (device and host use the same)
    cp_cores = []
    for ci in range(NCORES):
        parts = []
        for g in groups:
            r0 = g["rlo"] + ci * g["q"]
            r1 = min(g["rlo"] + min((ci + 1) * g["q"], g["m"]), g["rhi"])
            r0 = min(r0, r1)
            pv = ps[r0:r1]
            c = (np.float32(1.0) + pv) if g["flip"] else (np.float32(1.0) - pv)
            padded = np.full(g["T"] * P, dead, dtype=np.float32)
            padded[: len(c)] = c.astype(np.float32)
            parts.append(padded.astype(ml_dtypes.bfloat16))
        cp_cores.append(
            np.concatenate(parts) if parts else np.zeros(0, dtype=ml_dtypes.bfloat16)
        )

    # --- DMA chunk plan ----------------------------------------------------
    # chunk 0 (size _C0) rides the hot DMA; the rest cut at group boundaries,
    # subdivided to ~1030 cols, with a small final chunk to cut tail latency.
    c0sz = min(_C0, L)
    cuts = {0, c0sz, L}
    for g in groups:
        cuts.add(g["bc0"])
        cuts.add(g["bc0"] + g["ncol"])
    cuts = sorted(c for c in cuts if 0 <= c <= L)
    chunks = []
    for lo, hi in zip(cuts[:-1], cuts[1:]):
        span = hi - lo
        if span <= 0:
            continue
        k = max(1, -(-span // 1040))
        bnds = [lo + span * j // k for j in range(k + 1)]
        for j in range(k):
            chunks.append((bnds[j], bnds[j + 1]))
    # split the final chunk so the last-arriving piece is small
    if chunks and chunks[-1][1] - chunks[-1][0] > 700:
        lo, hi = chunks.pop()
        chunks.append((lo, hi - 384))
        chunks.append((hi - 384, hi))
    NCH = len(chunks)

    # --- arrival model -----------------------------------------------------
    # DMA order: hot (comb + chunk0), chunk1.., first two pre-block.
    arrival = [0.0] * NCH
    prev_h = _T0
    prev_d = 0.0
    hot_bytes = (combw + c0sz) * 2 * P
    plan_order = [-1] + list(range(1, NCH))  # -1 = hot (includes chunk 0)
    for oi, idx in enumerate(plan_order):
        h_end = prev_h + _HWDGE + (75.0 if oi == 2 else 0.0)
        prev_h = h_end
        if idx == -1:
            nbytes = hot_bytes
            elem = (combw + c0sz) * 2
        else:
            lo, hi = chunks[idx]
            nbytes = (hi - lo) * 2 * P
            elem = (hi - lo) * 2
        mult = 2.0 if elem < 512 else 1.0
        tr = max(nbytes * mult / 360.0, 7.0)
        d_end = max(h_end + _DGE, prev_d) + tr
        prev_d = d_end
        sem_t = d_end + _SEM_DMA
        if idx == -1:
            arrival[0] = sem_t
            comb_arrival = sem_t
        else:
            arrival[idx] = sem_t

    # --- work items --------------------------------------------------------
    items = []
    for gi, g in enumerate(groups):
        glo, ghi = g["bc0"], g["bc0"] + g["ncol"]
        for cidx, (lo, hi) in enumerate(chunks):
            s, e = max(lo, glo), min(hi, ghi)
            if s >= e:
                continue
            for tj in range(g["T"]):
                items.append(dict(gi=gi, tj=tj, cidx=cidx, c0=s - glo, nc=e - s))
    items.sort(key=lambda it: (arrival[it["cidx"]], it["gi"], it["tj"]))

    start = {
        "dve": comb_arrival + 30.0,
        "act": comb_arrival + 30.0,
        "pool": comb_arrival + 30.0 + _HUBER_POOL,
    }

    def schedule(T):
        """Greedy: slowest engine that still finishes each item by T.
        Returns (assignment, makespan)."""
        clock = dict(start)
        pend = [dict(it) for it in items]
        out = []
        i = 0
        while i < len(pend):
            it = pend[i]
            chosen = None
            for eng in ("pool", "act", "dve"):  # slowest first
                st = max(clock[eng], arrival[it["cidx"]])
                if st + _cost(eng, it["nc"]) <= T:
                    chosen = eng
                    break
            if chosen is None:
                chosen = min(
                    ("dve", "act", "pool"),
                    key=lambda e: max(clock[e], arrival[it["cidx"]])
                    + _cost(e, it["nc"]),
                )
            eng = chosen
            st = max(clock[eng], arrival[it["cidx"]])
            merged = dict(it)
            j = i + 1
            while j < len(pend):
                nx = pend[j]
                if (
                    nx["gi"] == merged["gi"]
                    and nx["tj"] == merged["tj"]
                    and nx["c0"] == merged["c0"] + merged["nc"]
                    and arrival[nx["cidx"]] <= st
                    and st + _cost(eng, merged["nc"] + nx["nc"]) <= T + 200.0
                ):
                    merged["nc"] += nx["nc"]
                    merged["cidx"] = max(merged["cidx"], nx["cidx"])
                    pend.pop(j)
                else:
                    break
            fin = st + _cost(eng, merged["nc"])
            clock[eng] = fin
            out.append((eng, merged))
            i += 1
        return out, max(clock.values())

    # bisect the smallest feasible target
    lo_t, hi_t = comb_arrival + 1000.0, comb_arrival + 30000.0
    best = schedule(hi_t)
    for _ in range(24):
        mid = 0.5 * (lo_t + hi_t)
        asg, mk = schedule(mid)
        if mk <= mid + 1.0:
            hi_t = mk
            best = (asg, mk)
        else:
            lo_t = mid
    assign, _mk = best if isinstance(best, tuple) else (best, 0.0)

    work = {"dve": [], "act": [], "pool": []}
    for eng, it in assign:
        work[eng].append(it)
    for eng in work:
        work[eng].sort(key=lambda it: (arrival[it["cidx"]], it["gi"], it["tj"]))
    nd = len(work["dve"])
    na = len(work["act"])
    npo = len(work["pool"])
    S = nd + na + npo + 1

    meta = dict(
        n=n, K=K, levels=levels, counts=counts.astype(np.int64), offs=offs,
        L=L, dead=dead, groups=groups, chunks=chunks, work=work,
        nd=nd, na=na, npo=npo, S=S, G=G, ch=ch, cht=cht, chp=cht * P,
        c0sz=c0sz, combw=combw, rt=int(cp_cores[0].shape[0]),
    )
    return meta, bcols, cp_cores, ps


def _shape_key(meta):
    gkey = tuple(
        (g["rlo"], g["rhi"], g["flip"], g["clo"], g["chi"], g["bc0"], g["T"])
        for g in meta["groups"]
    )
    ckey = tuple(meta["chunks"])
    wkey = tuple(
        (eng, tuple((it["gi"], it["tj"], it["cidx"], it["c0"], it["nc"])
                    for it in meta["work"][eng]))
        for eng in ("dve", "act", "pool")
    )
    return (meta["n"], meta["L"], meta["rt"], meta["cht"], meta["c0sz"], gkey,
            ckey, wkey)


@functools.lru_cache(maxsize=8)
def _build_program(key):
    """Raw Bass program: explicit per-engine streams and semaphores."""
    n, L, rt, cht, c0sz, gkey, ckey, wkey = key
    groups = [
        dict(rlo=a, rhi=b, flip=f, clo=c, chi=d, bc0=e, T=t, ncol=d - c)
        for (a, b, f, c, d, e, t) in gkey
    ]
    chunks = list(ckey)
    work = {eng: [dict(gi=gi, tj=tj, cidx=ci, c0=c0, nc=nc_)
                  for (gi, tj, ci, c0, nc_) in wl]
            for (eng, wl) in wkey}
    nd, na, npo = len(work["dve"]), len(work["act"]), len(work["pool"])
    S = nd + na + npo + 1
    G = sum(g["T"] for g in groups)
    combw = 2 * G + 2 * cht
    NCH = len(chunks)

    tbase = {}
    b = 0
    for gi, g in enumerate(groups):
        for tj in range(g["T"]):
            tbase[(gi, tj)] = b
            b += 1

    nc = bacc.Bacc("TRN2", enable_partition_id=False)

    fp32 = mybir.dt.float32
    bf16 = mybir.dt.bfloat16
    Alu = mybir.AluOpType
    Act = mybir.ActivationFunctionType

    # hot image: [cn | cp | pred | targ | chunk0], all bf16, per-partition
    d_hot = nc.dram_tensor("hot", [(combw + c0sz) * P], bf16, kind="ExternalInput")
    d_b = nc.dram_tensor("bcols", [max(L, 1)], bf16, kind="ExternalInput")
    d_acc = nc.dram_tensor("acc", [P, S], fp32, kind="ExternalOutput")

    # bt columns: [comb (combw) | broadcast layout (L)]
    bt = nc.alloc_sbuf_tensor("bt", [P, combw + max(L, 1)], bf16)
    combf = nc.alloc_sbuf_tensor("combf", [P, max(2 * G, 1)], fp32)
    acc = nc.alloc_sbuf_tensor("acc_t", [P, S], fp32)

    max_d = max([it["nc"] for it in work["dve"]], default=1)
    max_a = max([it["nc"] for it in work["act"]], default=1)
    max_p = max([it["nc"] for it in work["pool"]], default=1)
    scr_d = [nc.alloc_sbuf_tensor(f"scr_d{i}", [P, max_d], bf16) for i in range(2)]
    scr_a = [nc.alloc_sbuf_tensor(f"scr_a{i}", [P, max_a], bf16) for i in range(2)]
    scr_p = [nc.alloc_sbuf_tensor(f"scr_p{i}", [P, max_p], fp32) for i in range(2)]

    hd = nc.alloc_sbuf_tensor("hd", [P, cht], fp32)
    hr1 = nc.alloc_sbuf_tensor("hr1", [P, cht], fp32)
    he = nc.alloc_sbuf_tensor("he", [P, cht], fp32)
    hr2 = nc.alloc_sbuf_tensor("hr2", [P, cht], fp32)
    hsq = nc.alloc_sbuf_tensor("hsq", [P, cht], fp32)
    hs1 = nc.alloc_sbuf_tensor("hs1", [P, cht], fp32)
    hs2 = nc.alloc_sbuf_tensor("hs2", [P, cht], fp32)

    s_ch = [nc.alloc_semaphore(f"s_ch{i}") for i in range(max(NCH, 1))]
    s_cv = nc.alloc_semaphore("s_cv")
    s_dve = nc.alloc_semaphore("s_dve")
    s_act = nc.alloc_semaphore("s_act")
    s_pool = nc.alloc_semaphore("s_pool")
    s_out = nc.alloc_semaphore("s_out")

    cn_col = lambda gi, tj: combf[:, tbase[(gi, tj)] : tbase[(gi, tj)] + 1]
    cp_col = lambda gi, tj: combf[:, G + tbase[(gi, tj)] : G + tbase[(gi, tj)] + 1]
    pts = bt[:, 2 * G : 2 * G + cht]
    tts = bt[:, 2 * G + cht : 2 * G + 2 * cht]

    def bcol(layout_col):
        return combw + layout_col

    def bcast_dma(sync_eng, cidx):
        lo, hi = chunks[cidx]
        src = bass.AP(tensor=d_b[:].tensor, offset=lo, ap=[[0, P], [1, hi - lo]])
        sync_eng.dma_start(out=bt[:, bcol(lo) : bcol(hi)], in_=src).then_inc(
            s_ch[cidx], 16
        )

    # --- pre-barrier DMAs: hot (comb + chunk0), then chunk 1 ---------------
    nc.sync.dma_start(
        out=bt[:, 0 : combw + c0sz],
        in_=d_hot[:].rearrange("(p t) -> p t", p=P),
    ).then_inc(s_ch[0], 16)
    if NCH > 1:
        bcast_dma(nc.sync, 1)

    def emit_stream(eng, wl, scr, mk_inst, done_sem, first_extra=None,
                    pre_waited=()):
        waited = set(pre_waited)
        last = None
        if first_extra is not None:
            first_extra()
        for k, it in enumerate(wl):
            g = groups[it["gi"]]
            lo = g["bc0"] + it["c0"]
            hi = lo + it["nc"]
            for ci, (clo, chi) in enumerate(chunks):
                if clo < hi and lo < chi and ci not in waited:
                    eng.wait_ge(s_ch[ci], 16)
                    waited.add(ci)
            if 0 not in waited:
                eng.wait_ge(s_ch[0], 16)  # comb rides chunk 0's sem
                waited.add(0)
            last = mk_inst(eng, k, it, scr[k % 2], bcol(lo), bcol(hi))
        if last is not None:
            last.then_inc(done_sem, 1)
        return last

    with nc.Block() as block:

        @block.sync
        def _(sync):
            for ci in range(2, NCH):
                bcast_dma(sync, ci)
            sync.wait_ge(s_dve, 1)
            sync.wait_ge(s_act, 1)
            sync.wait_ge(s_pool, 1)
            with nc.allow_non_contiguous_dma(reason="small accumulator tile"):
                sync.dma_start(out=d_acc[:, :], in_=acc[:, :]).then_inc(s_out, 16)
            sync.wait_ge(s_out, 16)

        @block.vector
        def _(vector):
            def mk(eng, k, it, scr, lo, hi):
                return eng.tensor_scalar(
                    out=scr[:, : it["nc"]],
                    in0=bt[:, lo:hi],
                    scalar1=cn_col(it["gi"], it["tj"]),
                    scalar2=None,
                    op0=Alu.max,
                    op1=Alu.add,
                    accum_out=acc[:, k : k + 1],
                )
            def conv():
                # upconvert the bf16 row-constant columns to f32 scalars
                vector.wait_ge(s_ch[0], 16)
                vector.tensor_scalar(
                    out=combf[:, :], in0=bt[:, 0 : 2 * G], scalar1=0.0,
                    scalar2=None, op0=Alu.add,
                ).then_inc(s_cv, 1)
                vector.drain()
            if work["dve"]:
                emit_stream(vector, work["dve"], scr_d, mk, s_dve,
                            first_extra=conv, pre_waited=(0,))
            else:
                conv()
                vector.tensor_scalar(
                    out=scr_d[0][:, 0:1], in0=bt[:, 0:1], scalar1=0.0,
                    scalar2=None, op0=Alu.add,
                ).then_inc(s_dve, 1)

        @block.scalar
        def _(act):
            def mk(eng, k, it, scr, lo, hi):
                return eng.activation(
                    out=scr[:, : it["nc"]],
                    in_=bt[:, lo:hi],
                    func=Act.Relu,
                    bias=cp_col(it["gi"], it["tj"]),
                    scale=1.0,
                    accum_out=acc[:, nd + k : nd + k + 1],
                )

            def warmup():
                # no-wait dummy so the act-table load hoists into the DMA head
                act.activation(
                    out=scr_a[0][:, 0:1], in_=scr_a[1][:, 0:1], func=Act.Relu,
                    bias=0.0, scale=1.0,
                )
                act.wait_ge(s_cv, 1)  # f32 row-constant scalars ready
            if work["act"]:
                emit_stream(act, work["act"], scr_a, mk, s_act, first_extra=warmup)
            else:
                warmup()
                act.wait_ge(s_ch[0], 16)
                act.activation(
                    out=scr_a[0][:, 0:1], in_=bt[:, 0:1], func=Act.Relu,
                    bias=0.0, scale=1.0,
                ).then_inc(s_act, 1)

        @block.gpsimd
        def _(pool):
            pool.memset(acc[:, nd + na : S], 0.0)
            pool.wait_ge(s_ch[0], 16)
            pool.tensor_tensor(out=hd[:, :], in0=pts, in1=tts, op=Alu.subtract)
            pool.tensor_scalar(
                out=hr1[:, :], in0=hd[:, :], scalar1=1.0, scalar2=0.0,
                op0=Alu.subtract, op1=Alu.max,
            )
            pool.tensor_scalar(
                out=he[:, :], in0=hd[:, :], scalar1=-1.0, scalar2=1.0,
                op0=Alu.mult, op1=Alu.subtract,
            )
            pool.tensor_scalar(
                out=hr2[:, :], in0=he[:, :], scalar1=0.0, scalar2=None,
                op0=Alu.max,
            )
            pool.tensor_tensor(out=hsq[:, :], in0=hd[:, :], in1=hd[:, :], op=Alu.mult)
            pool.tensor_tensor(out=hs1[:, :], in0=hr1[:, :], in1=hr1[:, :], op=Alu.mult)
            pool.tensor_tensor(out=hs2[:, :], in0=hr2[:, :], in1=hr2[:, :], op=Alu.mult)
            pool.tensor_tensor(out=hsq[:, :], in0=hsq[:, :], in1=hs1[:, :], op=Alu.subtract)
            pool.tensor_tensor(out=hsq[:, :], in0=hsq[:, :], in1=hs2[:, :], op=Alu.subtract)
            hub = pool.tensor_reduce(
                out=acc[0:1, S - 1 : S], in_=hsq[:, :],
                axis=mybir.AxisListType.XYZWC, op=Alu.add,
            )
            if work["pool"]:
                pool.wait_ge(s_cv, 1)  # f32 row-constant scalars ready
                waited = {0}
                last = None
                for k, it in enumerate(work["pool"]):
                    g = groups[it["gi"]]
                    lo = g["bc0"] + it["c0"]
                    hi = lo + it["nc"]
                    for ci, (clo, chi) in enumerate(chunks):
                        if clo < hi and lo < chi and ci not in waited:
                            pool.wait_ge(s_ch[ci], 16)
                            waited.add(ci)
                    scr = scr_p[k % 2]
                    pool.tensor_scalar(
                        out=scr[:, : it["nc"]], in0=bt[:, bcol(lo) : bcol(hi)],
                        scalar1=cn_col(it["gi"], it["tj"]), scalar2=None,
                        op0=Alu.max,
                    )
                    last = pool.tensor_reduce(
                        out=acc[0 : 1, nd + na + k : nd + na + k + 1],
                        in_=scr[:, : it["nc"]],
                        axis=mybir.AxisListType.XYZWC, op=Alu.add,
                    )
                last.then_inc(s_pool, 1)
            else:
                hub.then_inc(s_pool, 1)

    nc.finalize()
    return nc


def _make_inputs(meta, bcols, cp_cores, predictions, targets):
    n = meta["n"]
    cht = meta["cht"]
    chp = meta["chp"]
    L = meta["L"]
    G = meta["G"]
    c0sz = meta["c0sz"]
    bf = ml_dtypes.bfloat16
    b_all = np.ascontiguousarray(
        bcols if L > 0 else np.zeros(1, dtype=np.float32), dtype=bf
    )
    in_maps = []
    for ci in range(NCORES):
        pc = np.zeros(chp, dtype=np.float32)
        tc_ = np.zeros(chp, dtype=np.float32)
        lo = ci * meta["ch"]
        hi = min((ci + 1) * meta["ch"], n)
        if hi > lo:
            pc[: hi - lo] = predictions[lo:hi]
            tc_[: hi - lo] = targets[lo:hi]
        cp = cp_cores[ci].astype(bf)
        cols = []
        if G > 0:
            cols.append((-cp.astype(np.float32)).astype(bf).reshape(G, P).T)
            cols.append(cp.reshape(G, P).T)
        cols.append(pc.astype(bf).reshape(cht, P).T)
        cols.append(tc_.astype(bf).reshape(cht, P).T)
        cols.append(np.broadcast_to(b_all[:c0sz], (P, c0sz)))
        hot2d = np.concatenate(cols, axis=1).astype(bf)  # [128, combw + c0sz]
        in_maps.append(
            {"hot": np.ascontiguousarray(hot2d.ravel()), "bcols": b_all}
        )
    return in_maps


def _gather(meta, cp_cores, results):
    """Combine per-core accumulators into the scalar loss (float64 host math)."""
    n = meta["n"]
    groups = meta["groups"]
    work = meta["work"]
    nd = meta["nd"]
    na = meta["na"]
    S = meta["S"]

    tbase = {}
    b = 0
    for gi, g in enumerate(groups):
        for tj in range(g["T"]):
            tbase[(gi, tj)] = b
            b += 1

    num = 0.0
    hub = 0.0
    for ci in range(NCORES):
        acc = results[ci]["acc"].astype(np.float64)
        cpv = cp_cores[ci].astype(np.float64)
        for k, it in enumerate(work["dve"]):
            tb = tbase[(it["gi"], it["tj"])]
            num += acc[:, k].sum() + it["nc"] * cpv[tb * P : (tb + 1) * P].sum()
        for k, it in enumerate(work["act"]):
            num += acc[:, nd + k].sum()
        for k, it in enumerate(work["pool"]):
            tb = tbase[(it["gi"], it["tj"])]
            num += acc[0, nd + na + k] + it["nc"] * cpv[tb * P : (tb + 1) * P].sum()
        hub += acc[0, S - 1]

    huber = 0.5 * hub / n

    counts = meta["counts"].astype(np.int64)
    csum = np.cumsum(counts)
    cnt = int(np.sum(counts[1:] * csum[:-1])) if len(counts) > 1 else 0
    ranking = num / float(np.float32(cnt)) if cnt > 0 else 0.0

    return np.float32(ALPHA * huber + BETA * ranking)


def _host_fallback(predictions, targets):
    """Safety net for input distributions the device plan is not built for
    (e.g. near-continuous targets). Exact O(n^2) evaluation, row-chunked."""
    p = predictions.astype(np.float64)
    t = targets.astype(np.float64)
    n = len(p)
    d = p - t
    ad = np.abs(d)
    huber = np.mean(np.where(ad < 1.0, 0.5 * d * d, ad - 0.5))
    num = 0.0
    cnt = 0
    step = 512
    for i0 in range(0, n, step):
        i1 = min(i0 + step, n)
        pd = p[i0:i1, None] - p[None, :]
        td = t[i0:i1, None] - t[None, :]
        sign = np.where(td > 0, 1.0, -1.0)
        idx = np.arange(n)
        mask = (td != 0) & (idx[i0:i1, None] < idx[None, :])
        hinge = np.maximum(0.0, 1.0 - sign * pd)
        num += hinge[mask].sum()
        cnt += int(mask.sum())
    ranking = num / float(np.float32(cnt)) if cnt > 0 else 0.0
    return np.float32(ALPHA * huber + BETA * ranking)


def kernel(predictions: np.ndarray, targets: np.ndarray) -> np.ndarray:
    predictions = np.asarray(predictions, dtype=np.float32)
    targets = np.asarray(targets, dtype=np.float32)

    nu = len(np.unique(targets))
    if nu > 16 or nu < 2 or predictions.shape[0] < NCORES * P:
        return np.array(_host_fallback(predictions, targets), dtype=np.float32)

    meta, bcols, cp_cores, _ps = _plan(targets, predictions)
    nc = _build_program(_shape_key(meta))
    in_maps = _make_inputs(meta, bcols, cp_cores, predictions, targets)
    res = run_bass_kernel_spmd(nc, in_maps, list(range(NCORES)))
    return np.array(_gather(meta, cp_cores, res.results), dtype=np.float32)
